# revision 23
# baseline (speedup 1.0000x reference)
"""Depth-aware 3x3 convolution on 8 Trainium2 NeuronCores (Bass, raw engine blocks).

out[b,o,h,w] = sum_{c,kh,kw} weight[o,c,kh,kw] * x[b,c,h+kh-1,w+kw-1]
                             * exp(-8.3*|depth[b,h,w] - depth[b,h+kh-1,w+kw-1]|)

Sharding: core = 2*b + (h >= 128); each core computes a [32, 128, 256] output
slab from a 130-row padded input frame (1-row halo from the host slice).

Datapath is bf16 (x, weight, sim, modulated product, output) with f32 depth
and f32 PSUM accumulation; the DVE modulation multiply runs in 2x perf mode
(all operands contiguous, 4B-aligned, pitch-256 pre-shifted on the host).

DMA strategy: the HWDGE ring processes DMAs ~serially (~0.26us fixed each +
transfer at the dest-partition-port rate), so traffic is split across BOTH
rings and batched at row-pair granularity (fat descriptors):
  ring A (SP):  d/w loads, sim stores, x3 pair-chunk loads, out stores,
                sim broadcasts for pairs 6-7
  ring B (ACT): sim broadcasts for pairs 0-5, issued after the copy whose
                pe_sem wait already implies the needed DVE progress (no ACT
                self-waits - an ACT DMA racing ACT's own compute crashes).

Per-core pipeline (pair = 2 tiles = 16 rows = 8192 px):
  A. sim: depth rows pixel-major [128, 258]x3 -> sub (DVE) -> |.| (DVE STT)
     -> exp (ACT, bf16) -> DRAM simd[9, 32768]
  B. main loop over 16 tiles of 2048 px:
     - DMA: x3 pair chunk [96, 18*256] bf16 (3 column-shift blocks)
     - DMA: 9 per-tap pair broadcasts simd[k, 8192px] -> [32, 8192] bf16
     - DVE: xm3 = x3[...] * simrep3[...]  (bf16 2x)  t=0,1,2
     - PE : psum[32, 2048] += w3[:, t].T @ xm3  (K=96, N=512 x4, bf16)
     - ACT: psum -> out_sb bf16; SP stores out.

The body sits in a per-engine hardware loop (`trips`) with a two-phase
leader-follower barrier and exact semaphore reset between trips; device time
is measured as the wall-clock slope between two trip counts.  Grading uses
trips=1.
"""
import sys

import numpy as np

sys.path.insert(0, "/opt/trn_rl_repo")

import concourse.bass as bass
import concourse.mybir as mybir
from concourse.bass_utils import run_bass_kernel_spmd

F32 = mybir.dt.float32
BF16 = mybir.dt.bfloat16
EXP = mybir.ActivationFunctionType.Exp

B, C, H, W = 4, 32, 256, 256
O = 32
ALPHA = 8.3
R = 128  # output rows per core
WP = W + 2  # padded width (depth frame only)
FR = R + 2  # frame rows per core
NPIX = R * W  # 32768
TROWS = 8  # rows per tile
TILE = TROWS * W  # 2048
NT = R // TROWS  # 16
NP = NT // 2  # 8 pairs
PCH = (2 * TROWS + 2) * W  # x3 pair chunk elems per partition (18 rows)
PPX = 2 * TILE  # pixels per pair
PGT = 3 * PPX  # simrep pair group (3 passes x 4096)
MMN = 512  # matmul free-dim chunk
QN = TILE // MMN  # 4
XMB = 4  # xm ring depth

# bc ring/slot assignment: pairs 0-5 on ACT ring, 6-7 on SP ring; 4 slots
BC_RING = ["a", "a", "a", "a", "a", "a", "s", "s"]
BC_KEYS = ["q0", "q1", "q2", "q3"]
_BC_SEM = {}
_cum = {k: 0 for k in BC_KEYS}
for _p in range(NP):
    _key = BC_KEYS[_p % 4]
    _cum[_key] += 288 if _p == 0 else 144
    _BC_SEM[_p] = (_key, _cum[_key])
BC_FINAL = dict(_cum)


def build_nc(trips=1):
    NO_BC = NO_X = NO_ST = NO_TT = NO_MM = NO_CP = False
    nc = bass.Bass("TRN2", target_bir_lowering=False, debug=False, num_devices=8)
    # x3: 3 column-shift blocks stacked on partitions, pitch-256 rows
    x3_in = nc.declare_dram_parameter("x3", [96, FR * W], BF16, isOutput=False)
    dp_in = nc.declare_dram_parameter("dp", [FR, WP], F32, isOutput=False)
    w3_in = nc.declare_dram_parameter("w3", [96, 96], BF16, isOutput=False)
    out_d = nc.declare_dram_parameter("out", [O, NPIX], BF16, isOutput=True)
    simd = nc.dram_tensor("simd", [9, NPIX], BF16)

    from contextlib import ExitStack

    ctx = ExitStack()
    with ctx:
        d_sb = ctx.enter_context(nc.sbuf_tensor([128, 3 * WP], F32))
        adiff9 = ctx.enter_context(nc.sbuf_tensor([128, 9 * W], F32))
        sim9 = ctx.enter_context(nc.sbuf_tensor([128, 9 * W], BF16))
        w3_sb = ctx.enter_context(nc.sbuf_tensor([96, 96], BF16))
        x3c = ctx.enter_context(nc.sbuf_tensor([96, 4 * PCH], BF16))
        simrep3 = ctx.enter_context(nc.sbuf_tensor([96, 4 * PGT], BF16))
        xm3 = ctx.enter_context(nc.sbuf_tensor([96, XMB * TILE], BF16))
        out_sb = ctx.enter_context(nc.sbuf_tensor([32, 4 * TILE], BF16))
        psum = ctx.enter_context(nc.psum_tensor([32, 2 * TILE], F32))
        ld_sem = ctx.enter_context(nc.semaphore("ld_sem"))
        sim_dve = ctx.enter_context(nc.semaphore("sim_dve"))
        act_exp = ctx.enter_context(nc.semaphore("act_exp"))
        sim_st = ctx.enter_context(nc.semaphore("sim_st"))
        x_q = [ctx.enter_context(nc.semaphore(f"x_q{r}")) for r in range(4)]
        bc_sems = {
            k: ctx.enter_context(nc.semaphore(f"bc_{k}")) for k in BC_KEYS
        }
        st_q = [ctx.enter_context(nc.semaphore(f"st_q{r}")) for r in range(4)]
        mod_sem = ctx.enter_context(nc.semaphore("mod_sem"))
        pe_sem = ctx.enter_context(nc.semaphore("pe_sem"))
        act_cp = ctx.enter_context(nc.semaphore("act_cp"))
        bar_g = ctx.enter_context(nc.semaphore("bar_g"))
        bar_r = ctx.enter_context(nc.semaphore("bar_r"))
        bar_a = ctx.enter_context(nc.semaphore("bar_a"))
        bar_r2 = ctx.enter_context(nc.semaphore("bar_r2"))
        block = ctx.enter_context(nc.Block())

        PIPE_SEMS = (
            [ld_sem, sim_dve, act_exp, sim_st, mod_sem, pe_sem, act_cp]
            + x_q + st_q + list(bc_sems.values())
        )

        def follower_barrier(eng):
            # two-phase: park on bar_r while SP resets pipe sems, then ack and
            # park on bar_r2 while SP resets bar_r.  All wait values are
            # trip-invariant; every sem returns to 0 each trip.
            eng.drain()
            eng.sem_inc(bar_g, 1)
            eng.wait_ge(bar_r, 1)
            eng.sem_inc(bar_a, 1)
            eng.wait_ge(bar_r2, 1)

        simd_r = simd.ap().rearrange("k (r w) -> k r w", r=R)

        def bc_pair(eng, p):
            # 9 per-tap broadcasts for pair p (2 tiles, 8192 px each)
            slot = p % 4
            sem = bc_sems[BC_KEYS[slot]]
            if NO_BC:
                eng.sem_inc(sem, 144)
                return
            for t in range(3):
                for j in range(3):
                    k = 3 * t + j
                    eng.dma_start(
                        simrep3[
                            32 * j : 32 * (j + 1),
                            slot * PGT + t * PPX : slot * PGT + (t + 1) * PPX,
                        ],
                        simd[k : k + 1, p * PPX : (p + 1) * PPX].to_broadcast(
                            (32, PPX)
                        ),
                    ).then_inc(sem, 16)

        @block.sync
        def _(sync: bass.BassEngine):
            with sync.Fori(0, trips):
                # startup loads: d (3 row-shifted views), w3
                for t in range(3):
                    sync.dma_start(
                        d_sb[:, t * WP : (t + 1) * WP], dp_in[t : t + 128, :]
                    ).then_inc(ld_sem, 16)
                sync.dma_start(w3_sb[:], w3_in[:]).then_inc(ld_sem, 16)
                # x3 pair chunks 0-3 (buffers free at trip start)
                for p in range(4):
                    if NO_X:
                        sync.sem_inc(x_q[p % 4], 16)
                    else:
                        sync.dma_start(
                            x3c[:, (p % 4) * PCH : (p % 4 + 1) * PCH],
                            x3_in[:, p * 2 * TROWS * W : p * 2 * TROWS * W + PCH],
                        ).then_inc(x_q[p % 4], 16)
                # sim stores (one per tap, gated per t-group)
                for k in range(9):
                    sync.wait_ge(act_exp, 3 * (k // 3) + 3)
                    sync.dma_start(
                        simd_r[k], sim9[:, k * W : (k + 1) * W]
                    ).then_inc(sim_st, 16)
                # main loop: stores first (loosest waits), then SP-side
                # broadcasts, then x3 loads (tightest waits)
                for i in range(NT):
                    if i >= 1 and NO_ST:
                        sync.sem_inc(st_q[(i - 1) % 4], 16)
                    elif i >= 1:
                        sync.wait_ge(act_cp, i)
                        sync.dma_start(
                            out_d[:, (i - 1) * TILE : i * TILE],
                            out_sb[:, ((i - 1) % 4) * TILE : ((i - 1) % 4 + 1) * TILE],
                        ).then_inc(st_q[(i - 1) % 4], 16)
                    if i == 6:
                        sync.wait_ge(mod_sem, 18)
                        bc_pair(sync, 6)
                    if i == 8:
                        sync.wait_ge(mod_sem, 24)
                        bc_pair(sync, 7)
                    if i % 2 == 0:
                        p = i // 2 + 4
                        if p < NP and NO_X:
                            sync.sem_inc(x_q[p % 4], 16)
                        elif p < NP:
                            sync.wait_ge(mod_sem, 6 * (p - 4) + 6)
                            sync.dma_start(
                                x3c[:, (p % 4) * PCH : (p % 4 + 1) * PCH],
                                x3_in[
                                    :, p * 2 * TROWS * W : p * 2 * TROWS * W + PCH
                                ],
                            ).then_inc(x_q[p % 4], 16)
                sync.wait_ge(act_cp, NT)
                if NO_ST:
                    sync.sem_inc(st_q[(NT - 1) % 4], 16)
                else:
                    sync.dma_start(
                        out_d[:, (NT - 1) * TILE :],
                        out_sb[:, ((NT - 1) % 4) * TILE : ((NT - 1) % 4 + 1) * TILE],
                    ).then_inc(st_q[(NT - 1) % 4], 16)
                # ---- trip barrier: leader ----
                sync.wait_ge(ld_sem, 64)
                sync.wait_ge(sim_st, 144)
                for r in range(4):
                    sync.wait_ge(x_q[r], 32)
                    sync.wait_ge(st_q[r], 64)
                for k, v in BC_FINAL.items():
                    if v:
                        sync.wait_ge(bc_sems[k], v)
                sync.wait_ge(bar_g, 3)
                for sem in PIPE_SEMS:
                    sync.sem_clear(sem)
                sync.sem_clear(bar_g)
                sync.sem_clear(bar_r2)
                sync.sem_inc(bar_r, 1)
                sync.wait_ge(bar_a, 3)
                sync.sem_clear(bar_r)
                sync.sem_clear(bar_a)
                sync.sem_inc(bar_r2, 1)

        @block.vector
        def _(vector):
            with vector.Fori(0, trips):
                # sim phase: grouped diff + abs per t (j via AP dims)
                vector.wait_ge(ld_sem, 64)
                for t in range(3):
                    dc = d_sb[:, WP + 1 : WP + 1 + W]
                    dc3 = bass.AP(dc.tensor, dc.offset, [dc.ap[0], [0, 3], [1, W]])
                    dk = d_sb[:, t * WP : t * WP + W]
                    dk3 = bass.AP(dk.tensor, dk.offset, [dk.ap[0], [1, 3], [1, W]])
                    vector.tensor_sub(
                        adiff9[:, 3 * t * W : (3 * t + 3) * W].rearrange(
                            "p (j w) -> p j w", j=3
                        ),
                        dc3,
                        dk3,
                    )
                    vector.drain()
                    vector.scalar_tensor_tensor(
                        adiff9[:, 3 * t * W : (3 * t + 3) * W],
                        adiff9[:, 3 * t * W : (3 * t + 3) * W],
                        -1.0,
                        adiff9[:, 3 * t * W : (3 * t + 3) * W],
                        op0=mybir.AluOpType.mult,
                        op1=mybir.AluOpType.max,
                    ).then_inc(sim_dve, 3)
                # modulation loop
                for i in range(NT):
                    p = i // 2
                    slot = p % 4
                    vector.wait_ge(x_q[slot], 16 * (p // 4 + 1))
                    if p > 0:
                        bk, bv = _BC_SEM[p]
                        vector.wait_ge(bc_sems[bk], bv)
                    for t in range(3):
                        s = 3 * i + t
                        sb = s % XMB
                        if i < 2:
                            vector.wait_ge(bc_sems["q0"], 144 * i + 48 * (t + 1))
                        if s >= XMB:
                            vector.wait_ge(pe_sem, s - XMB + 1)
                        if NO_TT:
                            vector.sem_inc(mod_sem, 1)
                            continue
                        vector.tensor_mul(
                            xm3[:, sb * TILE : (sb + 1) * TILE],
                            x3c[
                                :,
                                slot * PCH
                                + ((i % 2) * TROWS + t) * W : slot * PCH
                                + ((i % 2) * TROWS + t) * W
                                + TILE,
                            ],
                            simrep3[
                                :,
                                slot * PGT
                                + t * PPX
                                + (i % 2) * TILE : slot * PGT
                                + t * PPX
                                + (i % 2) * TILE
                                + TILE,
                            ],
                        ).then_inc(mod_sem, 1)
                follower_barrier(vector)

        @block.tensor
        def _(tensor):
            with tensor.Fori(0, trips):
                tensor.wait_ge(ld_sem, 64)
                for i in range(NT):
                    pb = i % 2
                    if i >= 2:
                        tensor.wait_ge(act_cp, i - 1)
                    for t in range(3):
                        s = 3 * i + t
                        sb = s % XMB
                        tensor.wait_ge(mod_sem, s + 1)
                        if NO_MM:
                            tensor.sem_inc(pe_sem, 1)
                            continue
                        for q in range(QN):
                            mm = tensor.matmul(
                                psum[
                                    :, pb * TILE + q * MMN : pb * TILE + (q + 1) * MMN
                                ],
                                w3_sb[:, 32 * t : 32 * (t + 1)],
                                xm3[:, sb * TILE + q * MMN : sb * TILE + (q + 1) * MMN],
                                start=(t == 0),
                                stop=(t == 2),
                            )
                            if q == QN - 1:
                                mm.then_inc(pe_sem, 1)
                follower_barrier(tensor)

        @block.scalar
        def _(scalar):
            with scalar.Fori(0, trips):
                # exp per t-group (bf16 out)
                for t in range(3):
                    scalar.wait_ge(sim_dve, 3 * t + 3)
                    scalar.activation(
                        sim9[:, 3 * t * W : (3 * t + 3) * W],
                        adiff9[:, 3 * t * W : (3 * t + 3) * W],
                        EXP,
                        scale=-ALPHA,
                    ).then_inc(act_exp, 3)
                # pair-0 broadcasts fine-grained (per tile-pass) so the
                # first TT can start as soon as tap rows land
                if NO_BC:
                    scalar.sem_inc(bc_sems["q0"], 288)
                else:
                    for half in range(2):
                        for t in range(3):
                            if half == 0:
                                scalar.wait_ge(sim_st, 48 * (t + 1))
                            for j in range(3):
                                k = 3 * t + j
                                scalar.dma_start(
                                    simrep3[
                                        32 * j : 32 * (j + 1),
                                        t * PPX
                                        + half * TILE : t * PPX
                                        + half * TILE
                                        + TILE,
                                    ],
                                    simd[
                                        k : k + 1, half * TILE : (half + 1) * TILE
                                    ].to_broadcast((32, TILE)),
                                ).then_inc(bc_sems["q0"], 16)
                # ACT-ring broadcasts for pairs 1-3
                scalar.wait_ge(sim_st, 144)
                for p in range(1, 4):
                    bc_pair(scalar, p)
                # psum -> sbuf copies; bc pairs 2-5 ride behind the copy whose
                # pe_sem wait implies the DVE progress they need
                for i in range(NT):
                    pb = i % 2
                    scalar.wait_ge(pe_sem, 3 * i + 3)
                    if i >= 4:
                        scalar.wait_ge(st_q[i % 4], 16 * (i // 4))
                    if NO_CP:
                        scalar.sem_inc(act_cp, 1)
                    else:
                        scalar.copy(
                            out_sb[:, (i % 4) * TILE : (i % 4 + 1) * TILE],
                            psum[:, pb * TILE : (pb + 1) * TILE],
                        ).then_inc(act_cp, 1)
                    if i == 1:
                        scalar.wait_ge(mod_sem, 6)
                        bc_pair(scalar, 4)
                    if i == 3:
                        scalar.wait_ge(mod_sem, 12)
                        bc_pair(scalar, 5)
                follower_barrier(scalar)

    return nc


_NC_CACHE = {}


def _get_nc(trips=1):
    if trips not in _NC_CACHE:
        _NC_CACHE[trips] = build_nc(trips)
    return _NC_CACHE[trips]


def _prep_core(x, depth, core):
    import ml_dtypes

    b, half = core // 2, core % 2
    r0 = half * R
    # padded frame [C, FR, WP]: image rows r0-1 .. r0+R, zero-padded
    xpad = np.zeros((C, FR, WP), dtype=np.float32)
    dpad = np.zeros((FR, WP), dtype=np.float32)
    lo, hi = r0 - 1, r0 + R + 1
    slo, shi = max(lo, 0), min(hi, H)
    xpad[:, slo - lo : shi - lo, 1 : 1 + W] = x[b, :, slo:shi, :]
    dpad[slo - lo : shi - lo, 1 : 1 + W] = depth[b, 0, slo:shi, :]
    # x3: 3 column-shift blocks stacked on partitions, pitch-256 (pre-shifted)
    x3 = np.empty((3, C, FR, W), dtype=np.float32)
    x3[0] = xpad[:, :, 0:W]  # j=0: w-1
    x3[1] = xpad[:, :, 1 : 1 + W]  # j=1: w
    x3[2] = xpad[:, :, 2 : 2 + W]  # j=2: w+1
    return {
        "x3": x3.reshape(3 * C, FR * W).astype(ml_dtypes.bfloat16),
        "dp": dpad,
        "w3": None,  # filled by caller (shared)
    }


def _prep_inputs(x, depth, weight):
    import ml_dtypes

    x = np.ascontiguousarray(x, dtype=np.float32)
    depth = np.ascontiguousarray(depth, dtype=np.float32)
    weight = np.ascontiguousarray(weight, dtype=np.float32)
    # w3[32j + c, 32t + o] = weight[o, c, t, j]
    w3 = (
        np.transpose(weight, (3, 1, 2, 0))
        .reshape(96, 96)
        .astype(ml_dtypes.bfloat16)
        .copy()
    )
    in_maps = []
    for core in range(8):
        m = _prep_core(x, depth, core)
        m["w3"] = w3
        in_maps.append(m)
    return in_maps


def kernel(x, depth, weight):
    in_maps = _prep_inputs(x, depth, weight)
    nc = _get_nc(1)
    res = run_bass_kernel_spmd(nc, in_maps, list(range(8)))

    out = np.empty((B, O, H, W), dtype=np.float32)
    for core in range(8):
        b, half = core // 2, core % 2
        out[b, :, half * R : (half + 1) * R, :] = (
            res.results[core]["out"].astype(np.float32).reshape(O, R, W)
        )
    return out
